# revision 48
# baseline (speedup 1.0000x reference)
"""v6: transpose-free, pair-fused, rep-pipelined build.

HW 418us (marginal over 12 back-to-back execs) vs v2 baseline 564us.
Sim (CoreSim cost model): 314us single-shot, 253us marginal/rep.

Design:
- scores computed directly in transposed (k x q) layout per key-block c:
  ST = kT_c^T @ qT[:, c*128:], exp writes ET straight from PSUM. No
  E->ET DMA transposes at all (v2 had 128 of them: ~74us SP issue +
  ~65us shared-DMA time). ET stored causally packed (36 blocks).
- causal mask folded into the score matmul group: ident^T @ mneg adds
  -1e4 below the diagonal in PSUM, exp underflows masked entries to
  exactly 0 (removes 128 Pool ops + a dependency hop).
- V projection token-major (lhsT=QT block, rhs=W_v natural [in,out])
  written straight into the VCn v-slots: no v transposes.
- projections fused into the head-pair loop and *woven* between score
  chunks at instruction granularity (weave()): PE always has
  dependency-free matmuls while the Act exp ring drains, instead of
  stalling on the score-PSUM WAR.
- cross-rep pipelining: all tile pools are opened once (pool close/
  reopen inserts allocator watermarks that serialize reps); weight
  slots rotate w0/w1/w2 per rep so rep n+1 W_q lands in the slot
  freed earliest; W_o staged late into W_q ring slot under the last
  head; phase D shares the c512 PSUM ring; output staged via a
  dedicated small ring and stored from the Act queue. Next rep Q/W
  loads then prefetch during this rep tail: marginal/rep 295 -> 253us.
- engine balance: Act = exps + qT/kT/pghb copies; DVE = PSUM->SBUF
  copies (u192, v, S/Z1, ot) + stats; Pool = weight/Q bf16 staging,
  ctx layernorm, outh; SP = all loads + QT/CT/pgn/outcT transposes.
- 1/sqrt on DVE via bit-trick + one Newton step (no act-table flips).
"""
import sys
sys.path.insert(0, '/opt/trn_rl_repo')
import numpy as np
from contextlib import ExitStack
from concourse import bass, mybir, bacc
import concourse.tile as tile
from concourse.masks import make_lower_triangular, make_identity

F32 = mybir.dt.float32
BF16 = mybir.dt.bfloat16
I32 = mybir.dt.int32
AF = mybir.ActivationFunctionType
ALU = mybir.AluOpType

L, D, H, DK = 1024, 1024, 16, 64
NT = 8
NPAIR = 8
LN_EPS = 1e-5
# causal-packed ET block offsets: block c holds q-cols [c*128, L)
OFFC = [0, 8, 15, 21, 26, 30, 33, 35]
NBLK = 36
LAST_HEAD_INTERLEAVE = False


def build(scale=0.125, bias_scale=0.1, reps=1):
    nc = bacc.Bacc(None, target_bir_lowering=False)

    dQ = nc.dram_tensor("Q", [L, D], F32, kind="ExternalInput")
    dCtx = nc.dram_tensor("ctx", [L, D], F32, kind="ExternalInput")
    dWq = nc.dram_tensor("W_q", [D, D], F32, kind="ExternalInput")
    dWk = nc.dram_tensor("W_k", [D, D], F32, kind="ExternalInput")
    dWv = nc.dram_tensor("W_v", [D, D], F32, kind="ExternalInput")
    dWo = nc.dram_tensor("W_o", [D, D], F32, kind="ExternalInput")
    dM = nc.dram_tensor("bilinear", [DK, DK], F32, kind="ExternalInput")
    dOut = nc.dram_tensor("out", [L, D], F32, kind="ExternalOutput")

    def mm(out, lhsT, rhs, start, stop, col0=0, **kw):
        n = rhs.shape[-1]
        assert out.shape[-1] == n
        j = 0
        while j < n:
            e = min(n, j + 512 - ((col0 + j) % 512))
            nc.tensor.matmul(out[..., j:e], lhsT, rhs[..., j:e],
                             start=start, stop=stop, **kw)
            j = e

    with ExitStack() as top:
        tc = top.enter_context(tile.TileContext(nc))
        singles = top.enter_context(tc.tile_pool(name="singles", bufs=1))
        persist = top.enter_context(tc.tile_pool(name="persist", bufs=1))

        # additive causal mask for the transposed (k x q) diagonal block:
        # -1e4 on the strict lower triangle (k > q), accumulated into the
        # score PSUM via ident^T @ mneg so exp underflows to exactly 0
        mneg = singles.tile([128, 128], BF16)
        make_lower_triangular(nc, mneg, val=-10000.0, diag=False)
        ones_col = singles.tile([128, 1], BF16)
        nc.vector.memset(ones_col, 1.0)
        m_f32 = singles.tile([64, DK], F32)
        nc.sync.dma_start(out=m_f32, in_=dM[:])
        eps_ln = singles.tile([128, 1], F32)
        nc.vector.memset(eps_ln, LN_EPS)
        magic = singles.tile([128, 16], I32)
        nc.vector.memset(magic, 0x5f3759df)
        ident = singles.tile([128, 128], BF16)
        make_identity(nc, ident)
        # m^T on-chip: pad m to [64,128] bf16, XBAR-transpose, take rows 0:64
        m_pad = singles.tile([64, 128], BF16)
        nc.vector.memset(m_pad, 0.0)
        m_sb = m_pad[:, 0:DK]
        nc.vector.tensor_copy(m_sb, m_f32)
        mt_full = singles.tile([128, DK], BF16)
        nc.sync.dma_start_transpose(mt_full, m_pad)
        mt_sb = mt_full[0:64, :]

        QT = persist.tile([128, NT, L], BF16)
        # VCn[:, hp, c, :] = [v_h0 | C_h0 | v_h1 | C_h1] (64 cols each)
        VCn = persist.tile([128, NPAIR, NT, 256], BF16)
        CT = persist.tile([128, NPAIR, L], BF16)
        outcT = persist.tile([128, NPAIR, NT, 128], BF16)

        def frsqrt(y, x, pool, tag):
            # y = 1/sqrt(x) on DVE only (no act tables): bit-trick seed +
            # one Newton step, max rel err ~1.8e-3.
            n = x.shape[-1]
            yi = pool.tile([128, n], I32, tag=tag + "i")
            nc.vector.tensor_scalar(yi, x.bitcast(I32), 1, None,
                                    ALU.arith_shift_right)
            nc.vector.tensor_tensor(y.bitcast(I32), magic[:, 0:n], yi,
                                    ALU.subtract)
            t1 = pool.tile([128, n], F32, tag=tag + "t")
            nc.vector.tensor_mul(t1, y, y)
            nc.vector.scalar_tensor_tensor(t1, x, -0.5, t1, ALU.mult, ALU.mult)
            nc.vector.tensor_scalar(t1, t1, 1.5, None, ALU.add)
            nc.vector.tensor_mul(y, y, t1)

        wpool = top.enter_context(tc.tile_pool(name="wpool", bufs=1))
        loadA = top.enter_context(tc.tile_pool(name="loadA", bufs=2))
        statsA = top.enter_context(tc.tile_pool(name="statsA", bufs=4))
        wload = top.enter_context(tc.tile_pool(name="wload", bufs=2))
        qk_pool = top.enter_context(tc.tile_pool(name="qk", bufs=3))
        et_pool = top.enter_context(tc.tile_pool(name="et", bufs=2))
        scr_pool = top.enter_context(tc.tile_pool(name="scr", bufs=1))
        pair_pool = top.enter_context(tc.tile_pool(name="pair", bufs=2))
        small_pool = top.enter_context(tc.tile_pool(name="small", bufs=2))
        u_pool = top.enter_context(tc.tile_pool(name="u", bufs=2))
        stat_pool = top.enter_context(tc.tile_pool(name="stat", bufs=3))
        ot_pool = top.enter_context(tc.tile_pool(name="otp", bufs=2))
        ps_c = top.enter_context(tc.tile_pool(name="ps_c", bufs=4,
                                              space="PSUM"))
        ps_pg = top.enter_context(tc.tile_pool(name="ps_pg", bufs=1,
                                               space="PSUM"))
        ps_sm = top.enter_context(tc.tile_pool(name="ps_sm", bufs=2,
                                               space="PSUM"))

        for _rep in range(reps):
            # ---- phase A: loads, Q transpose, ctx layernorm ----------------
            # DMA priority order on the shared DMA engines: W_q (Act queue)
            # ~ Q (SP) -> W_k (SP) -> ctx (SP) -> W_v (SP, issued late in
            # program order) -> W_o. CT transposes ride the DVE queue.
            wb_ = _rep % 3
            wqb = wpool.tile([128, NT, D], BF16, tag="w%d" % wb_)
            wkb = wpool.tile([128, NT, D], BF16, tag="w%d" % ((wb_ + 1) % 3))
            wvb = wpool.tile([128, NT, D], BF16, tag="w%d" % ((wb_ + 2) % 3))
            for dt_ in range(NT):
                wf = wload.tile([128, D], F32, tag="wf")
                nc.sync.dma_start(out=wf,
                                  in_=dWq[dt_ * 128:(dt_ + 1) * 128, :])
                nc.gpsimd.tensor_copy(wqb[:, dt_, :], wf)
            for t in range(NT):
                qf = loadA.tile([128, D], F32, tag="ld")
                nc.sync.dma_start(out=qf, in_=dQ[t * 128:(t + 1) * 128, :])
                qb_ = loadA.tile([128, D], BF16, tag="b16")
                nc.gpsimd.tensor_copy(qb_, qf)
                nc.sync.dma_start_transpose(
                    QT[:, :, t * 128:(t + 1) * 128], qb_)
            for dt_ in range(NT):
                wf = wload.tile([128, D], F32, tag="wf")
                nc.sync.dma_start(out=wf,
                                  in_=dWk[dt_ * 128:(dt_ + 1) * 128, :])
                nc.vector.tensor_copy(wkb[:, dt_, :], wf)
            # ctx load + per-head layernorm -> CT (transposed) + VCn C-slots
            for t in range(NT):
                cf = loadA.tile([128, D], F32, tag="ld")
                nc.sync.dma_start(out=cf, in_=dCtx[t * 128:(t + 1) * 128, :])
                cv = cf.rearrange("p (h e) -> p h e", h=H)
                sx = statsA.tile([128, H], F32, tag="sx")
                sxx = statsA.tile([128, H], F32, tag="sxx")
                x2 = loadA.tile([128, D], F32, tag="ld")
                nc.gpsimd.tensor_mul(x2, cf, cf)
                nc.vector.reduce_sum(sx, cv, axis=mybir.AxisListType.X)
                nc.vector.reduce_sum(sxx, x2.rearrange("p (h e) -> p h e", h=H),
                                     axis=mybir.AxisListType.X)
                mu = statsA.tile([128, H], F32, tag="mu")
                nc.scalar.mul(mu, sx, 1.0 / DK)
                var = statsA.tile([128, H], F32, tag="var")
                nc.vector.scalar_tensor_tensor(var, mu, 1.0, mu, ALU.mult,
                                               ALU.mult)
                nc.vector.tensor_scalar(var, var, -1.0, None, ALU.mult)
                ex2 = statsA.tile([128, H], F32, tag="ex2")
                nc.scalar.mul(ex2, sxx, 1.0 / DK)
                nc.vector.tensor_add(var, var, ex2)
                sd = statsA.tile([128, H], F32, tag="sd")
                nc.scalar.activation(sd, var, AF.Sqrt, bias=eps_ln)
                rs = statsA.tile([128, H], F32, tag="rs")
                nc.vector.reciprocal(rs, sd)
                cm = loadA.tile([128, D], F32, tag="ld")
                cmv = cm.rearrange("p (h e) -> p h e", h=H)
                nc.gpsimd.tensor_tensor(
                    cmv, cv, mu[:, :, None].to_broadcast([128, H, DK]),
                    ALU.subtract)
                Ctmp = loadA.tile([128, H, DK], BF16, tag="b16")
                nc.vector.tensor_tensor(
                    Ctmp, cmv,
                    rs[:, :, None].to_broadcast([128, H, DK]), ALU.mult)
                nc.sync.dma_start_transpose(
                    CT[:, :, t * 128:(t + 1) * 128],
                    Ctmp.rearrange("p h e -> p (h e)"))
                nc.vector.tensor_copy(
                    VCn[:, :, t, :].rearrange("p a (g x) -> p a g x",
                                              g=2)[:, :, :, 64:128],
                    Ctmp.rearrange("p (a g) e -> p a g e", g=2))
            # W_v loads issued here (SP reaches this point ~30us in, after
            # Q/ctx), staged on Pool; first needed by projv(0)/pg(0)
            for dt_ in range(NT):
                wf = wload.tile([128, D], F32, tag="wf")
                nc.sync.dma_start(out=wf,
                                  in_=dWv[dt_ * 128:(dt_ + 1) * 128, :])
                nc.gpsimd.tensor_copy(wvb[:, dt_, :], wf)

            # ---- per-pair fused projection + attention ---------------------
            st = {}

            def ETs(ET, c, q0, q1):
                # packed causal slice of block c, q-cols [q0, q1)
                base = OFFC[c] * 128 - c * 128
                return ET[:, base + q0:base + q1]

            def weave(primary, cover):
                # emit primary (Act-gated score chunks) with cover (PE-dense
                # work) distributed evenly between them, so PE always has
                # non-Act-dependent matmuls while the exp PSUM ring drains
                n, m = len(primary), len(cover)
                ci = 0
                for i, p in enumerate(primary):
                    p()
                    want = (m * (i + 1)) // max(n, 1)
                    while ci < want:
                        cover[ci]()
                        ci += 1
                while ci < m:
                    cover[ci]()
                    ci += 1

            def proj_qk_thunks(hp):
                # Q/K projections for pair hp -> transposed [dim, token]
                qTp = qk_pool.tile([128, L], BF16, tag="qT")
                kTp = qk_pool.tile([128, L], BF16, tag="kT")
                st[("qk", hp)] = (qTp, kTp)
                out = []
                for wsrc, dst in ((wqb, qTp), (wkb, kTp)):
                    for ch in range(2):
                        def t(wsrc=wsrc, dst=dst, ch=ch):
                            ps = ps_c.tile([128, 512], F32, tag="c512")
                            for dt_ in range(NT):
                                nc.tensor.matmul(
                                    ps, wsrc[:, dt_, hp * 128:(hp + 1) * 128],
                                    QT[:, dt_, ch * 512:(ch + 1) * 512],
                                    start=(dt_ == 0), stop=(dt_ == NT - 1))
                            nc.scalar.copy(dst[:, ch * 512:(ch + 1) * 512],
                                           ps)
                        out.append(t)
                return out

            def proj_v_thunks(hp):
                # V token-major straight into VCn v-slots
                out = []
                for ch in range(2):
                    def t(ch=ch):
                        ps = ps_c.tile([128, 512], F32, tag="c512")
                        for tq in range(4):
                            tb = ch * 4 + tq
                            for dt_ in range(NT):
                                nc.tensor.matmul(
                                    ps[:, tq * 128:(tq + 1) * 128],
                                    QT[:, dt_, tb * 128:(tb + 1) * 128],
                                    wvb[:, dt_, hp * 128:(hp + 1) * 128],
                                    start=(dt_ == 0), stop=(dt_ == NT - 1))
                        nc.vector.tensor_copy(
                            VCn[:, hp, ch * 4:(ch + 1) * 4, :].rearrange(
                                "p c (g y) -> p c g y", g=2)[:, :, :, 0:64],
                            ps.rearrange("p (c g x) -> p c g x", c=4, g=2))
                    out.append(t)
                return out

            def scores_thunks(h):
                hp, hl = h // 2, (h % 2) * 64
                qTp, kTp = st[("qk", hp)]
                if h % 2 == 0:
                    se = stat_pool.tile([128, 16], F32, tag="se")
                    e2 = stat_pool.tile([128, 16], F32, tag="e2")
                    st[("stats", hp)] = (se, e2)
                # scores directly transposed: per key-block c,
                # ST[k, q] = kT_c^T @ qT for q >= c*128 (causal)
                ET = et_pool.tile([128, NBLK * 128], BF16, tag="et")
                st[("et", h)] = ET
                out = []
                for c in range(NT):
                    qlo = c * 128
                    for cl in range(qlo, L, 512):
                        def t(c=c, cl=cl, qlo=qlo):
                            cw = min(512, L - cl)
                            diag = cl == qlo
                            ps = ps_c.tile([128, 512], F32, tag="c512")
                            nc.tensor.matmul(ps[:, 0:cw],
                                             kTp[hl:hl + 64, qlo:qlo + 128],
                                             qTp[hl:hl + 64, cl:cl + cw],
                                             start=True, stop=not diag)
                            if diag:
                                # causal: add -1e4 below the diagonal so the
                                # exp flushes masked entries to exactly 0
                                nc.tensor.matmul(ps[:, 0:128], ident, mneg,
                                                 start=False, stop=True)
                            nc.scalar.activation(ETs(ET, c, cl, cl + cw),
                                                 ps[:, 0:cw], AF.Exp,
                                                 scale=float(scale))
                        out.append(t)
                return out

            def pg_thunks(h):
                hp, hl = h // 2, (h % 2) * 64
                ET = st.pop(("et", h))
                se, e2 = st[("stats", hp)]
                c0 = (h % 2) * NT
                hl2 = (h % 2) * 128
                pghb = pair_pool.tile([128, L], BF16, tag="pghb")
                pgn = pair_pool.tile([128, NT, 128], BF16, tag="pgn")
                st[("pgn", h)] = pgn
                u192 = u_pool.tile([128, NT, 192], BF16, tag="u192")
                st[("u", h)] = u192
                if h % 2 == 0:
                    bn2 = stat_pool.tile([128, 16], F32, tag="bn2")
                    eb = stat_pool.tile([128, 16], F32, tag="eb")
                    st[("bstats", hp)] = (bn2, eb)
                else:
                    bn2, eb = st[("bstats", hp)]
                rhs192 = small_pool.tile([128, 192], BF16, tag="rhs")
                out = []

                def t_gram(qb):
                    psE = ps_sm.tile([128, 160], F32, tag="sm")
                    for c in range(qb + 1):
                        blk = ETs(ET, c, qb * 128, (qb + 1) * 128)
                        nc.tensor.matmul(psE[:, 0:128], blk, blk,
                                         start=(c == 0), stop=(c == qb))
                    scrE = scr_pool.tile([128, 128], BF16, tag="scrE")
                    nc.vector.scalar_tensor_tensor(
                        scrE, psE[:, 0:128], 1.0, ident, ALU.mult, ALU.mult,
                        accum_out=e2[:, c0 + qb:c0 + qb + 1])
                    # se group runs after the e2 group closes: PSUM
                    # accumulation state is per (partition, bank)
                    for c in range(qb + 1):
                        nc.tensor.matmul(
                            psE[:, 128:129],
                            ETs(ET, c, qb * 128, (qb + 1) * 128),
                            ones_col, start=(c == 0), stop=(c == qb))
                    nc.vector.tensor_copy(se[:, c0 + qb:c0 + qb + 1],
                                          psE[:, 128:129])
                grams = [lambda qb=qb: t_gram(qb) for qb in range(NT)]
                out = list(grams)
                late = []

                def t_pgL():
                    pgL = ps_pg.tile([128, 512], F32, tag="pgL")
                    st[("pgL", h)] = pgL
                    for c in range(4):
                        nc.tensor.matmul(pgL[:, c * 128:512],
                                         VCn[:, hp, c, hl2:hl2 + 128],
                                         ETs(ET, c, c * 128, 512),
                                         start=(c == 0), stop=(c == 3),
                                         skip_group_check=True)
                    nc.scalar.copy(pghb[:, 0:512], pgL)
                late.append(t_pgL)

                def t_pgR():
                    pgR = ps_pg.tile([128, 512], F32, tag="pgR")
                    for c in range(NT):
                        lo = max(0, (c - 4) * 128)
                        nc.tensor.matmul(pgR[:, lo:512],
                                         VCn[:, hp, c, hl2:hl2 + 128],
                                         ETs(ET, c, 512 + lo, L),
                                         start=(c == 0), stop=(c == NT - 1),
                                         skip_group_check=True)
                    nc.scalar.copy(pghb[:, 512:L], pgR)
                    # pgn[:, c, 0:64] = P^T, pgn[:, c, 64:128] = G^T
                    nc.sync.dma_start_transpose(pgn, pghb)
                late.append(t_pgR)

                def t_small():
                    psS = ps_sm.tile([64, DK], F32, tag="sm")
                    for c in range(NT):
                        cs = VCn[:, hp, c, hl2 + 64:hl2 + 128]
                        nc.tensor.matmul(psS, cs, cs,
                                         start=(c == 0), stop=(c == NT - 1))
                    S_sb = small_pool.tile([64, DK], BF16, tag="S")
                    nc.vector.tensor_copy(S_sb, psS)
                    psMS = ps_sm.tile([64, DK], F32, tag="sm")
                    nc.tensor.matmul(psMS, mt_sb, S_sb, start=True, stop=True)
                    psZ1 = ps_sm.tile([64, DK], F32, tag="sm")
                    for c in range(NT):
                        nc.tensor.matmul(psZ1,
                                         VCn[:, hp, c, hl2 + 64:hl2 + 128],
                                         VCn[:, hp, c, hl2:hl2 + 64],
                                         start=(c == 0), stop=(c == NT - 1))
                    Z1_sb = small_pool.tile([64, DK], BF16, tag="Z1")
                    nc.vector.tensor_copy(Z1_sb, psZ1)
                    psZ2 = ps_sm.tile([64, DK], F32, tag="sm")
                    nc.tensor.matmul(psZ2, mt_sb, Z1_sb, start=True, stop=True)
                    nc.gpsimd.tensor_copy(rhs192[hl:hl + 64, 0:64], m_sb)
                    nc.vector.tensor_copy(rhs192[hl:hl + 64, 64:128], psMS)
                    nc.vector.tensor_copy(rhs192[hl:hl + 64, 128:192], psZ2)
                out.append(t_small)

                def t_uwp(qb):
                    # u192[:, qb, 0:64]=u  64:128]=w  128:192]=p2
                    uwp = ps_sm.tile([128, 192], F32, tag="sm")
                    nc.tensor.matmul(
                        uwp, CT[hl:hl + 64, hp, qb * 128:(qb + 1) * 128],
                        rhs192[hl:hl + 64, :], start=True, stop=True)
                    nc.vector.tensor_copy(u192[:, qb, :], uwp)
                for qb in range(NT):
                    out.append(lambda qb=qb: t_uwp(qb))

                def t_bn2():
                    # bn2 = sum_e u*w per (q, qb), on Pool
                    scrb = u_pool.tile([128, NT, 64], BF16, tag="scr3")
                    nc.gpsimd.tensor_mul(scrb, u192[:, :, 0:64],
                                         u192[:, :, 64:128])
                    nc.vector.reduce_sum(bn2[:, c0:c0 + NT], scrb,
                                         axis=mybir.AxisListType.X)
                out.append(t_bn2)
                return {"grams": grams, "mid": out[NT:], "pgLR": late}

            def pg_list(g):
                return g["grams"] + g["mid"] + g["pgLR"]

            def stage_pair_finish(hp):
                pgn0 = st.pop(("pgn", 2 * hp))
                pgn1 = st.pop(("pgn", 2 * hp + 1))
                se, e2 = st.pop(("stats", hp))
                bn2, eb = st.pop(("bstats", hp))
                u0 = st.pop(("u", 2 * hp))
                u1 = st.pop(("u", 2 * hp + 1))
                # eb = <exp-row, g-row> per (q, qb), on Pool
                for h, u192, pgn in ((2 * hp, u0, pgn0),
                                     (2 * hp + 1, u1, pgn1)):
                    c0 = (h % 2) * NT
                    scre = u_pool.tile([128, NT, 64], BF16, tag="scr3")
                    nc.gpsimd.tensor_mul(scre, u192[:, :, 0:64],
                                         pgn[:, :, 64:128])
                    nc.vector.reduce_sum(eb[:, c0:c0 + NT], scre,
                                         axis=mybir.AxisListType.X)
                # ---- batched stats for both heads: [128,16] DVE ops ----
                c = float(bias_scale)
                g = stat_pool.tile([128, 16], F32, tag="g")
                nc.vector.reciprocal(g, se)
                rb = stat_pool.tile([128, 16], F32, tag="rb")
                frsqrt(rb, bn2, stat_pool, "rb")
                t1 = stat_pool.tile([128, 16], F32, tag="t1")
                nc.vector.tensor_mul(t1, e2, g)
                nc.vector.tensor_mul(t1, t1, g)
                t2 = stat_pool.tile([128, 16], F32, tag="t2")
                nc.vector.tensor_mul(t2, eb, g)
                nc.vector.tensor_mul(t2, t2, rb)
                an2 = stat_pool.tile([128, 16], F32, tag="an2")
                nc.vector.scalar_tensor_tensor(an2, t2, 2.0 * c, t1,
                                               ALU.mult, ALU.add)
                nc.vector.tensor_scalar(an2, an2, c * c, None, ALU.add)
                ra = stat_pool.tile([128, 16], F32, tag="ra")
                frsqrt(ra, an2, stat_pool, "ra")
                alpha = stat_pool.tile([128, 16], F32, tag="alpha")
                nc.vector.tensor_mul(alpha, g, ra)
                beta = stat_pool.tile([128, 16], F32, tag="beta")
                nc.vector.scalar_tensor_tensor(beta, rb, c, ra,
                                               ALU.mult, ALU.mult)
                # ---- apply: outh = alpha*p1 + beta*p2, on Pool ----------
                outh = pair_pool.tile([128, NT, 128], BF16, tag="pghb")
                for h, u192, pgn in ((2 * hp, u0, pgn0),
                                     (2 * hp + 1, u1, pgn1)):
                    hl, c0 = (h % 2) * 64, (h % 2) * NT
                    al_b = alpha[:, c0:c0 + NT][:, :, None].to_broadcast(
                        [128, NT, 64])
                    be_b = beta[:, c0:c0 + NT][:, :, None].to_broadcast(
                        [128, NT, 64])
                    nc.gpsimd.tensor_tensor(outh[:, :, hl:hl + 64],
                                            pgn[:, :, 0:64], al_b,
                                            ALU.mult)
                    nc.gpsimd.tensor_tensor(u192[:, :, 128:192],
                                            u192[:, :, 128:192], be_b,
                                            ALU.mult)
                    nc.gpsimd.tensor_add(outh[:, :, hl:hl + 64],
                                         outh[:, :, hl:hl + 64],
                                         u192[:, :, 128:192])
                nc.sync.dma_start_transpose(outcT[:, hp, :, :], outh)

            for th in (proj_qk_thunks(0) + proj_qk_thunks(1)
                       + proj_qk_thunks(2)):
                th()
            weave(scores_thunks(0), proj_v_thunks(0))
            for h in range(1, H):
                hp = h // 2
                pgm = pg_thunks(h - 1)
                if h == H - 1 and LAST_HEAD_INTERLEAVE:
                    # pgLR(14) first: pgL/pgR ring slots must be written in
                    # emission order before pg(15) reuses them mid-weave
                    cover = pgm["pgLR"] + pgm["grams"] + pgm["mid"]
                else:
                    cover = pg_list(pgm)
                if h % 2 == 1:
                    if hp + 1 < NPAIR:
                        cover += proj_v_thunks(hp + 1)
                else:
                    cover.append(lambda hp=hp: stage_pair_finish(hp - 1))
                    if hp + 2 < NPAIR:
                        cover += proj_qk_thunks(hp + 2)
                if h == H - 1:
                    # W_o staged into W_q's ring slot (its readers end at
                    # pair 7's projection); loads on the SP queue (idle by
                    # now), staging woven under the last scores
                    Wo_b = wpool.tile([128, NT, D], BF16,
                                      tag="w%d" % wb_)

                    def wo_stage(dt_):
                        wf = wload.tile([128, D], F32, tag="wf")
                        nc.sync.dma_start(
                            out=wf, in_=dWo[dt_ * 128:(dt_ + 1) * 128, :])
                        if dt_ % 2 == 0:
                            nc.gpsimd.tensor_copy(Wo_b[:, dt_, :], wf)
                        else:
                            nc.vector.tensor_copy(Wo_b[:, dt_, :], wf)
                    cover += [lambda dt_=dt_: wo_stage(dt_)
                              for dt_ in range(NT)]
                if h == H - 1 and LAST_HEAD_INTERLEAVE:
                    # last head: emit its score rows with gram(qb) right
                    # behind row qb's diagonal chunk (gram qb needs rows
                    # <= qb only), so the pair-7 finish chain starts as the
                    # last exp lands instead of after all of S(15)+pg(15)
                    sth = scores_thunks(h)
                    pg15 = pg_thunks(h)
                    cover += pg15["mid"]
                    rows = []
                    i = 0
                    for c in range(NT):
                        n_ch = len(range(c * 128, L, 512))
                        rows.append(sth[i:i + n_ch])
                        i += n_ch
                    prim = []
                    for c in range(NT):
                        prim.append(rows[c][0])
                        prim.append(pg15["grams"][c])
                        prim += rows[c][1:]
                        if c == 3:
                            prim.append(pg15["pgLR"][0])
                    prim.append(pg15["pgLR"][1])
                    weave(prim, cover)
                else:
                    weave(scores_thunks(h), cover)
            if not LAST_HEAD_INTERLEAVE:
                for th in pg_list(pg_thunks(H - 1)):
                    th()
            stage_pair_finish(NPAIR - 1)

            # ---- phase D: output projection (shares the c512 PSUM ring and
            # loadA staging so the next rep's prologue pipelines right in) --
            for t in range(NT):
                for ch in range(2):
                    ps = ps_c.tile([128, 512], F32, tag="c512")
                    for hp in range(NPAIR):
                        nc.tensor.matmul(
                            ps, outcT[:, hp, t, :],
                            Wo_b[:, hp, ch * 512:(ch + 1) * 512],
                            start=(hp == 0), stop=(hp == NPAIR - 1))
                    ot = ot_pool.tile([128, 512], F32, tag="ot")
                    nc.vector.tensor_copy(ot, ps)
                    nc.scalar.dma_start(
                        out=dOut[t * 128:(t + 1) * 128,
                                 ch * 512:(ch + 1) * 512], in_=ot)

    return nc


_BUILT = {}


def _get_built(scale, bias_scale, reps=1):
    key = (round(float(scale), 9), round(float(bias_scale), 9), int(reps))
    if key not in _BUILT:
        nc = build(scale=float(scale), bias_scale=float(bias_scale), reps=reps)
        nc.finalize()
        _BUILT[key] = nc
    return _BUILT[key]


def kernel(**inputs):
    from concourse.bass_utils import run_bass_kernel_spmd

    Q = np.asarray(inputs["Q"], dtype=np.float32)
    ctx = np.asarray(inputs["ctx"], dtype=np.float32)
    B = Q.shape[0]
    assert B == 8 and Q.shape[1:] == (L, D)
    weights = {k: np.ascontiguousarray(np.asarray(inputs[k], dtype=np.float32))
               for k in ("W_q", "W_k", "W_v", "W_o", "bilinear")}
    nc = _get_built(inputs["scale"], inputs["bias_scale"])
    in_maps = []
    for b in range(B):
        m = {"Q": np.ascontiguousarray(Q[b]), "ctx": np.ascontiguousarray(ctx[b])}
        m.update(weights)
        in_maps.append(m)
    res = run_bass_kernel_spmd(nc, in_maps, list(range(B)))
    return np.stack([res.results[b]["out"] for b in range(B)]).astype(np.float32)
